# revision 4
# baseline (speedup 1.0000x reference)
"""Trainium2 Bass kernel for the 2-layer-GCN + MLP-head model
(nn_Base_single_embedding_89859305767623).

Contract: kernel(**inputs) takes the FULL unsharded inputs and returns the
FULL output tuple (s_ci, s_ci, s_ci, h_ci, h_ci) matching reference().

Strategy (8 NeuronCores, SPMD, 3 launches with host exchange between the
two GCN aggregation layers). Nodes are dst-sharded 6250/core and permuted
into 56 tiles x 128 slots per core, load-balancing per-tile edge counts.

  L1: x_g0T = lrelu(Wg0.T @ dxT + bg0); t1T = Wg1.T @ x_g0T  -> out raw.
      Host: ys1[n,:] = dinv[n] * t1[n,:]  (scale + transpose + un-permute).
  L2: gather ys1[src] rows per edge via GPSIMD dma_gather (int16 indices,
      two 25k-row windows); segment-sum via one-hot matmuls into PSUM
      (aggT[f, slot] += sum_e dinv_dst[e] * onehot[slot](e) * G[e, f]);
      x_g1T = lrelu(aggT + bg1); t2T = Wg2.T @ x_g1T -> out raw.
      Host: ys2 = dinv * t2.
  L3: same aggregation on ys2 -> x_g2T; dense head (slab-streamed):
      x_dT/x_c1T/x_c2T, hT = lrelu(Wf.T @ concat + bf), sT,
      yT = sigmoid(Wl2.T @ sT + bl2).

  All matmuls run in float32r (tf32-like, full PE rate at these shapes);
  the one-hot selection matrices carry the dst-side 1/sqrt(deg) factor,
  the src-side factor is folded into the gather tables on the host.
"""

import os
import sys
import types

import numpy as np

# ---------------------------------------------------------------------------
# environment shims (self-contained; no sibling imports)
# ---------------------------------------------------------------------------


def _install_axon_hook_shim():
    """bass_utils wants antenv.axon_hooks for NTFF profiling under axon; the
    agent image lacks it. Provide it and register the ctypes-based hook from
    trn_agent_boot when available."""
    if "antenv.axon_hooks" in sys.modules:
        return
    mod = types.ModuleType("antenv.axon_hooks")
    hook = [None]
    mod.set_axon_ntff_profile_hook = lambda h: hook.__setitem__(0, h)
    mod.get_axon_ntff_profile_hook = lambda: hook[0]
    sys.modules["antenv.axon_hooks"] = mod
    try:
        import antenv

        antenv.axon_hooks = mod
    except ImportError:
        pass
    try:
        from trn_agent_boot.trn_boot import _ntff_profile_via_ctypes

        mod.set_axon_ntff_profile_hook(
            _ntff_profile_via_ctypes("/opt/axon/libaxon_pjrt.so")
        )
    except Exception:
        pass


_install_axon_hook_shim()

import bass_rust  # noqa: E402
import concourse.bacc as bacc  # noqa: E402
import concourse.mybir as mybir  # noqa: E402
from concourse import bass_utils  # noqa: E402
from concourse.tile import TileContext  # noqa: E402
import concourse.tile as _tile_mod  # noqa: E402
from concourse.vector_clock import ScopedClock  # noqa: E402

# artifact upload is unavailable in this container; keep trace processing local
bass_utils.upload_artifacts = lambda tmpdir: "local://" + tmpdir

F32 = mybir.dt.float32
F32R = mybir.dt.float32r
I16 = mybir.dt.int16
LRELU = mybir.ActivationFunctionType.Lrelu

_wsplit_ctr = [0]


def _split_multi_waits(nc, max_waits=1):
    """The walrus build in this env rejects instructions carrying more than
    one sync wait. Hoist extras onto injected same-engine EventSemaphore
    instructions placed immediately before (same engine program order =>
    identical semantics)."""
    for f in nc.m.functions:
        for bb in f.blocks:
            insts = bb.instructions
            if not any(
                inst.sync_info is not None and len(inst.sync_info.on_wait) > max_waits
                for inst in insts
            ):
                continue
            new = []
            for inst in insts:
                si = inst.sync_info
                if si is not None and len(si.on_wait) > max_waits:
                    waits = list(si.on_wait)
                    keep, extra = waits[-max_waits:], waits[:-max_waits]
                    for w in extra:
                        _wsplit_ctr[0] += 1
                        ev = mybir.InstEventSemaphore(
                            name=f"wsplit_{_wsplit_ctr[0]}", ins=[], outs=[]
                        )
                        ev.engine = inst.engine
                        ev.sync_info = bass_rust.SyncInfo(on_wait=[w], on_update=[])
                        new.append(ev)
                    si.on_wait = keep
                new.append(inst)
            bb.instructions = new


def _drain_and_barrier(self, tick_clock, wait_clock):
    """Tail drain emitting one wait per Drain instruction (same wait limit)."""
    nc = self.nc
    drain_inst = nc.sync.drain()
    wait_clock.add_sem_waits(
        drain_inst.ins, ScopedClock({None: tick_clock.global_clock})
    )
    si = drain_inst.ins.sync_info
    waits = list(si.on_wait) if si is not None else []
    if len(waits) > 1:
        si.on_wait = waits[:1]
        for w in waits[1:]:
            extra = nc.sync.drain()
            esi = extra.ins.sync_info
            if esi is None:
                extra.ins.sync_info = bass_rust.SyncInfo(on_wait=[w], on_update=[])
            else:
                esi.on_wait = [w]
    nc.all_engine_barrier()
    assert self.sems is not None
    popped = nc._tile_sem_poison_stack.pop()
    assert popped is self._sem_poison
    nc.clear_and_free_semaphores(list(self.sems.allocated().values()))
    nc.all_engine_barrier()


_tile_mod.TileContext._drain_and_barrier = _drain_and_barrier

# ---------------------------------------------------------------------------
# problem constants (hardcoded per the harness contract)
# ---------------------------------------------------------------------------

N_NODES = 50000
N_CORES = 8
NPC = N_NODES // N_CORES  # 6250 nodes per core
D_DISC = 64
D_CONT = 13
EMB = 256
HID = 512
SLOPE = 0.01

T_TILES = 56                # dst tiles per core (128 slots each)
SLOTS = T_TILES * 128       # 7168 slots per core
HALF = N_NODES // 2         # int16 gather window split
MAX_CHUNKS_PER_CALL = 8     # 8*128 = 1024 rows per dma_gather (SWDGE ring cap)
SLAB = 512                  # node-slot slab for dense matmuls
N_SLABS = SLOTS // SLAB

# results of the last kernel() invocation, for the test harness
LAST_RESULTS = []


# ---------------------------------------------------------------------------
# host-side graph planning
# ---------------------------------------------------------------------------


def _plan_core(src, dst, dinv, core):
    """Assign this core's dst nodes to tiles/slots balancing lo/hi edge
    counts, and build per-tile lo/hi edge arrays."""
    lo_base = core * NPC
    sel = (dst >= lo_base) & (dst < lo_base + NPC)
    es = src[sel]
    ed = dst[sel] - lo_base  # local node id
    is_hi = es >= HALF

    deg_lo = np.bincount(ed[~is_hi], minlength=NPC)
    deg_hi = np.bincount(ed[is_hi], minlength=NPC)

    order = np.argsort(-(deg_lo + deg_hi), kind="stable")
    lo_sum = np.zeros(T_TILES, np.int64)
    hi_sum = np.zeros(T_TILES, np.int64)
    cnt = np.zeros(T_TILES, np.int64)
    tile_of_node = np.full(NPC, -1, np.int32)
    slot_in_tile = np.full(NPC, -1, np.int32)
    big = np.iinfo(np.int64).max
    for n in order:
        dl, dh = deg_lo[n], deg_hi[n]
        score = np.maximum(lo_sum + dl, hi_sum + dh) * 1000 + cnt
        score[cnt >= 128] = big
        t = int(np.argmin(score))
        tile_of_node[n] = t
        slot_in_tile[n] = cnt[t]
        cnt[t] += 1
        lo_sum[t] += dl
        hi_sum[t] += dh

    node_of_slot = np.full(SLOTS, -1, np.int64)
    node_of_slot[tile_of_node * 128 + slot_in_tile] = np.arange(NPC) + lo_base

    e_tile = tile_of_node[ed]
    e_slot = slot_in_tile[ed]
    e_dinv = dinv[ed + lo_base]

    per_tile = []
    for t in range(T_TILES):
        m = e_tile == t
        mlo = m & ~is_hi
        mhi = m & is_hi
        per_tile.append(
            (
                (es[mlo], e_slot[mlo], e_dinv[mlo]),
                (es[mhi] - HALF, e_slot[mhi], e_dinv[mhi]),
            )
        )
    return {"node_of_slot": node_of_slot, "per_tile": per_tile,
            "lo_max": int(lo_sum.max()), "hi_max": int(hi_sum.max())}


def _build_plan(edge_index):
    src = np.asarray(edge_index[0], np.int64)
    dst = np.asarray(edge_index[1], np.int64)
    loops = np.arange(N_NODES, dtype=np.int64)
    src = np.concatenate([src, loops])
    dst = np.concatenate([dst, loops])

    deg = np.bincount(dst, minlength=N_NODES).astype(np.float64)
    dinv = (1.0 / np.sqrt(np.maximum(deg, 1e-12))).astype(np.float32)

    cores = [_plan_core(src, dst, dinv, c) for c in range(N_CORES)]
    k_lo = max((c["lo_max"] + 127) // 128 for c in cores)
    k_hi = max((c["hi_max"] + 127) // 128 for c in cores)

    def _calls(k):
        out = []
        k0 = 0
        while k0 < k:
            k1 = min(k0 + MAX_CHUNKS_PER_CALL, k)
            out.append((k0, k1))
            k0 = k1
        return out

    calls_lo = _calls(k_lo)
    calls_hi = _calls(k_hi)
    cpt = k_lo + k_hi
    C = T_TILES * cpt

    plans = []
    for c in range(N_CORES):
        dstpos = np.full((128, C), 999.0, np.float32)
        dinvd = np.zeros((128, C), np.float32)
        idx16 = np.zeros((128, C * 8), np.int16)

        for t in range(T_TILES):
            for half, (e_idx, e_slot, e_dinv), koff, kcalls in (
                (0, cores[c]["per_tile"][t][0], 0, calls_lo),
                (1, cores[c]["per_tile"][t][1], k_lo, calls_hi),
            ):
                m = len(e_idx)
                if m == 0:
                    continue
                j = np.arange(m)
                k = j // 128
                p = j % 128
                col = t * cpt + koff + k
                dstpos[p, col] = e_slot.astype(np.float32)
                dinvd[p, col] = e_dinv
                for (k0, k1) in kcalls:
                    in_call = (k >= k0) & (k < k1)
                    jc = j[in_call] - k0 * 128
                    colbase = (t * cpt + koff + k0) * 8
                    rows = (jc % 16).astype(np.int64)
                    colsw = colbase + jc // 16
                    idx16[rows, colsw] = e_idx[in_call].astype(np.int16)
        for r in range(1, 8):
            idx16[16 * r : 16 * r + 16] = idx16[:16]
        plans.append(
            {
                "node_of_slot": cores[c]["node_of_slot"],
                "dstpos": dstpos,
                "dinvd": dinvd,
                "idx16": idx16,
            }
        )
    return {
        "dinv": dinv,
        "k_lo": k_lo,
        "k_hi": k_hi,
        "calls_lo": calls_lo,
        "calls_hi": calls_hi,
        "cpt": cpt,
        "C": C,
        "cores": plans,
    }


# ---------------------------------------------------------------------------
# device kernel builders
# ---------------------------------------------------------------------------


def _bias_cols(b):
    """[F] -> [128, ceil(F/128)] bias layout (col m = b[128m:128m+128])."""
    b = np.asarray(b, np.float32).ravel()
    if b.size % 128 != 0:
        b = np.pad(b, (0, 128 - b.size % 128))
    return np.ascontiguousarray(b.reshape(-1, 128).T)


_IOTA = np.ascontiguousarray(
    np.broadcast_to(np.arange(128, dtype=np.float32)[None, :], (128, 128))
)


def _load_const(nc, pool, t_dram, shape, dtype, name):
    tile = pool.tile(shape, dtype, name=name)
    nc.sync.dma_start(out=tile[:], in_=t_dram[:])
    return tile


def _load_w_tiles(nc, pool, t_w, k_dim, m_tiles, tag):
    """Load a [K, M] f32r weight dram tensor into [m][k] SBUF lhsT tiles."""
    out = []
    n_k = (k_dim + 127) // 128
    for m in range(m_tiles):
        row = []
        for k in range(n_k):
            kk = min(128, k_dim - k * 128)
            w = pool.tile([kk, 128], F32R, name=f"{tag}_{m}_{k}")
            nc.sync.dma_start(
                out=w[:], in_=t_w[k * 128 : k * 128 + kk, m * 128 : (m + 1) * 128]
            )
            row.append(w)
        out.append(row)
    return out


def _emit_aggregation(nc, work, psum, plan, t_table, idx_t, dstpos_t, dinvd_t,
                      iota_t, bias_t, xg0, xg1):
    """Gather + one-hot-matmul aggregation over all tiles; writes
    lrelu(aggT + bias) into xg0/xg1 ([128, SLOTS] f32r, feature-major)."""
    k_lo, cpt = plan["k_lo"], plan["cpt"]
    win_lo = t_table[0:HALF, :]
    win_hi = t_table[HALF:N_NODES, :]
    n_chunks = plan["k_lo"] + plan["k_hi"]

    for t in range(T_TILES):
        acc0 = psum.tile([128, 128], F32, space="PSUM", tag="acc0", bufs=2,
                         name=f"acc0_{t}")
        acc1 = psum.tile([128, 128], F32, space="PSUM", tag="acc1", bufs=2,
                         name=f"acc1_{t}")
        ci = 0
        for koff, kcalls, win in (
            (0, plan["calls_lo"], win_lo),
            (k_lo, plan["calls_hi"], win_hi),
        ):
            for (k0, k1) in kcalls:
                nch = k1 - k0
                ni = nch * 128
                colbase = (t * cpt + koff + k0) * 8
                g = work.tile([128, nch * EMB], F32R, tag="g", bufs=3,
                              name=f"g_{t}_{koff}_{k0}")
                nc.gpsimd.dma_gather(
                    out_ap=g[:, : nch * EMB].rearrange("p (c e) -> p c e", e=EMB),
                    in_ap=win,
                    idxs_ap=idx_t[:, colbase : colbase + ni // 16],
                    num_idxs=ni,
                    num_idxs_reg=ni,
                    elem_size=EMB,
                )
                for k in range(k0, k1):
                    col = t * cpt + koff + k
                    s = work.tile([128, 128], F32R, tag="S", bufs=4,
                                  name=f"s_{t}_{koff}_{k}")
                    nc.vector.tensor_scalar(
                        out=s[:],
                        in0=iota_t[:],
                        scalar1=dstpos_t[:, col : col + 1],
                        scalar2=dinvd_t[:, col : col + 1],
                        op0=mybir.AluOpType.is_equal,
                        op1=mybir.AluOpType.mult,
                    )
                    gk = (k - k0) * EMB
                    first = ci == 0
                    last = ci == n_chunks - 1
                    nc.tensor.matmul(
                        out=acc0[:], lhsT=g[:, gk : gk + 128], rhs=s[:],
                        start=first, stop=last,
                    )
                    nc.tensor.matmul(
                        out=acc1[:], lhsT=g[:, gk + 128 : gk + 256], rhs=s[:],
                        start=first, stop=last,
                    )
                    ci += 1
        sl = slice(t * 128, (t + 1) * 128)
        nc.scalar.activation(xg0[:, sl], acc0[:], LRELU,
                             bias=bias_t[:, 0:1], alpha=SLOPE)
        nc.scalar.activation(xg1[:, sl], acc1[:], LRELU,
                             bias=bias_t[:, 1:2], alpha=SLOPE)


def _linear(nc, work, psum, w_tiles, rhs_aps, bias_t, func, n_out, out_cb,
            tagp, s):
    """out[m] = func(sum_i w_tiles[m][i].T @ rhs_aps[i] + bias[:, m]) for one
    slab; out delivered via out_cb(m, producer) where producer(out_ap) emits
    the finalizing op writing into out_ap."""
    for m in range(n_out):
        acc = psum.tile([128, SLAB], F32, space="PSUM", tag="dacc", bufs=3,
                        name=f"dacc_{tagp}_{s}_{m}")
        for i, rhs in enumerate(rhs_aps):
            nc.tensor.matmul(
                out=acc[:], lhsT=w_tiles[m][i][:], rhs=rhs,
                start=(i == 0), stop=(i == len(rhs_aps) - 1),
            )
        if func is None:
            out_cb(m, lambda ap: nc.vector.tensor_copy(out=ap, in_=acc[:]))
        else:
            out_cb(m, lambda ap: nc.scalar.activation(
                ap, acc[:], func, bias=bias_t[:, m : m + 1], alpha=SLOPE))


def _build_l1(plan):
    nc = bacc.Bacc(None, target_bir_lowering=False)
    t_dx = nc.dram_tensor("dxT", [D_DISC, SLOTS], F32R, kind="ExternalInput")
    t_wg0 = nc.dram_tensor("Wg0", [D_DISC, EMB], F32R, kind="ExternalInput")
    t_bg0 = nc.dram_tensor("bg0", [128, 2], F32, kind="ExternalInput")
    t_wg1 = nc.dram_tensor("Wg1", [EMB, EMB], F32R, kind="ExternalInput")
    t_out = nc.dram_tensor("t1T", [EMB, SLOTS], F32, kind="ExternalOutput")

    with TileContext(nc) as tc:
        with tc.tile_pool(name="sbuf", bufs=1) as pool, \
             tc.tile_pool(name="work", bufs=1) as work, \
             tc.tile_pool(name="psum", bufs=1, space="PSUM") as psum:
            dx = _load_const(nc, pool, t_dx, [D_DISC, SLOTS], F32R, "dx")
            bg0 = _load_const(nc, pool, t_bg0, [128, 2], F32, "bg0")
            w0 = _load_w_tiles(nc, pool, t_wg0, D_DISC, 2, "w0")
            w1 = _load_w_tiles(nc, pool, t_wg1, EMB, 2, "w1")
            xg0 = [pool.tile([128, SLOTS], F32R, name=f"xg0_{m}") for m in range(2)]
            for s in range(N_SLABS):
                sl = slice(s * SLAB, (s + 1) * SLAB)

                def cb1(m, producer, sl=sl):
                    producer(xg0[m][:, sl])

                _linear(nc, work, psum, w0, [dx[:, sl]], bg0, LRELU, 2, cb1,
                        "a", s)
            for s in range(N_SLABS):
                sl = slice(s * SLAB, (s + 1) * SLAB)

                def cb2(m, producer, sl=sl, s=s):
                    stage = work.tile([128, SLAB], F32, tag="stage", bufs=3,
                                      name=f"stage_{s}_{m}")
                    producer(stage[:])
                    nc.sync.dma_start(
                        out=t_out[m * 128 : (m + 1) * 128, sl], in_=stage[:]
                    )

                _linear(nc, work, psum, w1,
                        [xg0[0][:, sl], xg0[1][:, sl]], None, None, 2, cb2,
                        "b", s)
    nc.finalize()
    _split_multi_waits(nc)
    return nc


def _agg_inputs(nc, plan):
    C = plan["C"]
    t = {}
    t["table"] = None  # set by caller
    t["idx16"] = nc.dram_tensor("idx16", [128, C * 8], I16, kind="ExternalInput")
    t["dstpos"] = nc.dram_tensor("dstpos", [128, C], F32, kind="ExternalInput")
    t["dinvd"] = nc.dram_tensor("dinvd", [128, C], F32, kind="ExternalInput")
    t["iota"] = nc.dram_tensor("iota", [128, 128], F32, kind="ExternalInput")
    return t


def _load_agg(nc, pool, plan, t):
    C = plan["C"]
    idx = _load_const(nc, pool, t["idx16"], [128, C * 8], I16, "idx")
    dstpos = _load_const(nc, pool, t["dstpos"], [128, C], F32, "dstpos")
    dinvd = _load_const(nc, pool, t["dinvd"], [128, C], F32, "dinvd")
    iota = _load_const(nc, pool, t["iota"], [128, 128], F32, "iota")
    return idx, dstpos, dinvd, iota


def _build_l2(plan):
    nc = bacc.Bacc(None, target_bir_lowering=False)
    t_table = nc.dram_tensor("ys1", [N_NODES, EMB], F32R, kind="ExternalInput")
    ta = _agg_inputs(nc, plan)
    t_bg1 = nc.dram_tensor("bg1", [128, 2], F32, kind="ExternalInput")
    t_wg2 = nc.dram_tensor("Wg2", [EMB, EMB], F32R, kind="ExternalInput")
    t_out = nc.dram_tensor("t2T", [EMB, SLOTS], F32, kind="ExternalOutput")

    with TileContext(nc) as tc:
        with tc.tile_pool(name="sbuf", bufs=1) as pool, \
             tc.tile_pool(name="work", bufs=1) as work, \
             tc.tile_pool(name="psum", bufs=1, space="PSUM") as psum:
            idx, dstpos, dinvd, iota = _load_agg(nc, pool, plan, ta)
            bg1 = _load_const(nc, pool, t_bg1, [128, 2], F32, "bg1")
            w2 = _load_w_tiles(nc, pool, t_wg2, EMB, 2, "w2")
            xg = [pool.tile([128, SLOTS], F32R, name=f"xg_{m}") for m in range(2)]
            _emit_aggregation(nc, work, psum, plan, t_table, idx, dstpos,
                              dinvd, iota, bg1, xg[0], xg[1])
            for s in range(N_SLABS):
                sl = slice(s * SLAB, (s + 1) * SLAB)

                def cb(m, producer, sl=sl, s=s):
                    stage = work.tile([128, SLAB], F32, tag="stage", bufs=3,
                                      name=f"stage_{s}_{m}")
                    producer(stage[:])
                    nc.sync.dma_start(
                        out=t_out[m * 128 : (m + 1) * 128, sl], in_=stage[:]
                    )

                _linear(nc, work, psum, w2, [xg[0][:, sl], xg[1][:, sl]],
                        None, None, 2, cb, "p", s)
    nc.finalize()
    _split_multi_waits(nc)
    return nc


def _build_l3(plan):
    nc = bacc.Bacc(None, target_bir_lowering=False)
    t_table = nc.dram_tensor("ys2", [N_NODES, EMB], F32R, kind="ExternalInput")
    ta = _agg_inputs(nc, plan)
    t_bg2 = nc.dram_tensor("bg2", [128, 2], F32, kind="ExternalInput")
    t_dx = nc.dram_tensor("dxT", [D_DISC, SLOTS], F32R, kind="ExternalInput")
    t_c1 = nc.dram_tensor("c1T", [D_CONT, SLOTS], F32R, kind="ExternalInput")
    t_c2 = nc.dram_tensor("c2T", [D_CONT, SLOTS], F32R, kind="ExternalInput")
    t_wd = nc.dram_tensor("Wd", [D_DISC, EMB], F32R, kind="ExternalInput")
    t_bd = nc.dram_tensor("bd", [128, 2], F32, kind="ExternalInput")
    t_wc1 = nc.dram_tensor("Wc1", [D_CONT, EMB], F32R, kind="ExternalInput")
    t_bc1 = nc.dram_tensor("bc1", [128, 2], F32, kind="ExternalInput")
    t_wc2 = nc.dram_tensor("Wc2", [D_CONT, EMB], F32R, kind="ExternalInput")
    t_bc2 = nc.dram_tensor("bc2", [128, 2], F32, kind="ExternalInput")
    t_wf = nc.dram_tensor("Wf", [4 * EMB, HID], F32R, kind="ExternalInput")
    t_bf = nc.dram_tensor("bf", [128, 4], F32, kind="ExternalInput")
    t_wl1 = nc.dram_tensor("Wl1", [HID, HID // 2], F32R, kind="ExternalInput")
    t_bl1 = nc.dram_tensor("bl1", [128, 2], F32, kind="ExternalInput")
    t_wl2 = nc.dram_tensor("Wl2", [HID // 2, 1], F32R, kind="ExternalInput")
    t_bl2 = nc.dram_tensor("bl2", [1, 1], F32, kind="ExternalInput")
    t_h = nc.dram_tensor("hT", [HID, SLOTS], F32, kind="ExternalOutput")
    t_y = nc.dram_tensor("yT", [1, SLOTS], F32, kind="ExternalOutput")

    with TileContext(nc) as tc:
        with tc.tile_pool(name="sbuf", bufs=1) as pool, \
             tc.tile_pool(name="work", bufs=1) as work, \
             tc.tile_pool(name="psum", bufs=1, space="PSUM") as psum:
            idx, dstpos, dinvd, iota = _load_agg(nc, pool, plan, ta)
            bg2 = _load_const(nc, pool, t_bg2, [128, 2], F32, "bg2")
            bd = _load_const(nc, pool, t_bd, [128, 2], F32, "bd")
            bc1 = _load_const(nc, pool, t_bc1, [128, 2], F32, "bc1")
            bc2 = _load_const(nc, pool, t_bc2, [128, 2], F32, "bc2")
            bf = _load_const(nc, pool, t_bf, [128, 4], F32, "bf")
            bl1 = _load_const(nc, pool, t_bl1, [128, 2], F32, "bl1")
            bl2 = _load_const(nc, pool, t_bl2, [1, 1], F32, "bl2")

            wd = _load_w_tiles(nc, pool, t_wd, D_DISC, 2, "wd")
            wc1 = _load_w_tiles(nc, pool, t_wc1, D_CONT, 2, "wc1")
            wc2 = _load_w_tiles(nc, pool, t_wc2, D_CONT, 2, "wc2")
            wf = _load_w_tiles(nc, pool, t_wf, 4 * EMB, 4, "wf")
            wl1 = _load_w_tiles(nc, pool, t_wl1, HID, 2, "wl1")
            wl2 = []
            for k in range(2):
                w = pool.tile([128, 1], F32R, name=f"wl2_{k}")
                nc.sync.dma_start(out=w[:], in_=t_wl2[k * 128 : (k + 1) * 128, 0:1])
                wl2.append(w)

            xg = [pool.tile([128, SLOTS], F32R, name=f"xg_{m}") for m in range(2)]
            _emit_aggregation(nc, work, psum, plan, t_table, idx, dstpos,
                              dinvd, iota, bg2, xg[0], xg[1])

            for s in range(N_SLABS):
                sl = slice(s * SLAB, (s + 1) * SLAB)
                dxs = work.tile([D_DISC, SLAB], F32R, tag="dxs", bufs=2,
                                name=f"dxs_{s}")
                nc.sync.dma_start(out=dxs[:], in_=t_dx[:, sl])
                c1s = work.tile([D_CONT, SLAB], F32R, tag="c1s", bufs=2,
                                name=f"c1s_{s}")
                nc.sync.dma_start(out=c1s[:], in_=t_c1[:, sl])
                c2s = work.tile([D_CONT, SLAB], F32R, tag="c2s", bufs=2,
                                name=f"c2s_{s}")
                nc.sync.dma_start(out=c2s[:], in_=t_c2[:, sl])

                xparts = []
                for tagp, w_t, rhs, b_t in (
                    ("xd", wd, dxs, bd), ("xc1", wc1, c1s, bc1),
                    ("xc2", wc2, c2s, bc2),
                ):
                    outs = [
                        work.tile([128, SLAB], F32R, tag=f"{tagp}{m}", bufs=2,
                                  name=f"{tagp}_{s}_{m}")
                        for m in range(2)
                    ]

                    def cbx(m, producer, outs=outs):
                        producer(outs[m][:])

                    _linear(nc, work, psum, w_t, [rhs[:]], b_t, LRELU, 2, cbx,
                            tagp, s)
                    xparts.extend(outs)
                xparts.extend([xg[0][:, sl], xg[1][:, sl]])

                ht = [
                    work.tile([128, SLAB], F32R, tag=f"ht{m}", bufs=2,
                              name=f"ht_{s}_{m}")
                    for m in range(4)
                ]

                def cbh(m, producer, ht=ht, sl=sl):
                    producer(ht[m][:])
                    nc.sync.dma_start(
                        out=t_h[m * 128 : (m + 1) * 128, sl],
                        in_=ht[m][:].bitcast(F32),
                    )

                _linear(nc, work, psum, wf, xparts, bf, LRELU, 4, cbh, "h", s)

                st = [
                    work.tile([128, SLAB], F32R, tag=f"st{m}", bufs=2,
                              name=f"st_{s}_{m}")
                    for m in range(2)
                ]

                def cbs(m, producer, st=st):
                    producer(st[m][:])

                _linear(nc, work, psum, wl1, [ht[0][:], ht[1][:], ht[2][:],
                                              ht[3][:]], bl1, LRELU, 2, cbs,
                        "s", s)

                yacc = psum.tile([1, SLAB], F32, space="PSUM", tag="yacc",
                                 bufs=1, name=f"yacc_{s}")
                for k in range(2):
                    nc.tensor.matmul(out=yacc[:], lhsT=wl2[k][:], rhs=st[k][:],
                                     start=(k == 0), stop=(k == 1))
                ys = work.tile([1, SLAB], F32, tag="ystage", bufs=3,
                               name=f"ystage_{s}")
                nc.scalar.activation(ys[:], yacc[:],
                                     mybir.ActivationFunctionType.Sigmoid,
                                     bias=bl2[:, 0:1])
                nc.sync.dma_start(out=t_y[0:1, sl], in_=ys[:])
    nc.finalize()
    _split_multi_waits(nc)
    return nc


# ---------------------------------------------------------------------------
# launcher
# ---------------------------------------------------------------------------


def _run(nc, in_maps, tag):
    trace = bool(int(os.environ.get("GNN_TRACE", "0")))
    kw = {}
    if trace:
        kw["trace"] = True
        kw["trace_cores"] = [0]
        td = os.environ.get("GNN_TRACE_DIR")
        if td:
            kw["tmpdir"] = os.path.join(td, tag)
            os.makedirs(kw["tmpdir"], exist_ok=True)
    res = bass_utils.run_bass_kernel_spmd(
        nc, in_maps, core_ids=list(range(N_CORES)), **kw
    )
    LAST_RESULTS.append((tag, res))
    return res.results


def kernel(discrete_x, continous_x, Wd, bd, Wc1, bc1, Wc2, bc2,
           Wg0, bg0, Wg1, bg1, Wg2, bg2, Wf, bf, Wl1, bl1, Wl2, bl2,
           edge_index):
    LAST_RESULTS.clear()
    discrete_x = np.asarray(discrete_x, np.float32)
    continous_x = np.asarray(continous_x, np.float32)
    to32 = lambda a: np.ascontiguousarray(np.asarray(a, np.float32))

    plan = _build_plan(np.asarray(edge_index))
    dinv = plan["dinv"]

    dxT, c1T, c2T, valid = [], [], [], []
    for c in range(N_CORES):
        nos = plan["cores"][c]["node_of_slot"]
        v = nos >= 0
        dx = np.zeros((SLOTS, D_DISC), np.float32)
        dx[v] = discrete_x[nos[v]]
        cx = np.zeros((SLOTS, 3 * D_CONT), np.float32)
        cx[v] = continous_x[nos[v]]
        dxT.append(np.ascontiguousarray(dx.T))
        c1T.append(np.ascontiguousarray(cx[:, :D_CONT].T))
        c2T.append(np.ascontiguousarray(cx[:, D_CONT : 2 * D_CONT].T))
        valid.append(v)

    # ---- L1 ----
    nc1 = _build_l1(plan)
    in1 = [
        {"dxT": dxT[c], "Wg0": to32(Wg0), "bg0": _bias_cols(bg0),
         "Wg1": to32(Wg1)}
        for c in range(N_CORES)
    ]
    r1 = _run(nc1, in1, "l1")

    ys1 = np.zeros((N_NODES, EMB), np.float32)
    for c in range(N_CORES):
        nos = plan["cores"][c]["node_of_slot"]
        v = valid[c]
        ys1[nos[v]] = r1[c]["t1T"].T[v]
    ys1 *= dinv[:, None]

    # ---- L2 ----
    nc2 = _build_l2(plan)
    in2 = [
        {"ys1": ys1, "idx16": plan["cores"][c]["idx16"],
         "dstpos": plan["cores"][c]["dstpos"],
         "dinvd": plan["cores"][c]["dinvd"],
         "iota": _IOTA, "bg1": _bias_cols(bg1), "Wg2": to32(Wg2)}
        for c in range(N_CORES)
    ]
    r2 = _run(nc2, in2, "l2")

    ys2 = np.zeros((N_NODES, EMB), np.float32)
    for c in range(N_CORES):
        nos = plan["cores"][c]["node_of_slot"]
        v = valid[c]
        ys2[nos[v]] = r2[c]["t2T"].T[v]
    ys2 *= dinv[:, None]

    # ---- L3 ----
    nc3 = _build_l3(plan)
    in3 = [
        {"ys2": ys2, "idx16": plan["cores"][c]["idx16"],
         "dstpos": plan["cores"][c]["dstpos"],
         "dinvd": plan["cores"][c]["dinvd"],
         "iota": _IOTA, "bg2": _bias_cols(bg2),
         "dxT": dxT[c], "c1T": c1T[c], "c2T": c2T[c],
         "Wd": to32(Wd), "bd": _bias_cols(bd),
         "Wc1": to32(Wc1), "bc1": _bias_cols(bc1),
         "Wc2": to32(Wc2), "bc2": _bias_cols(bc2),
         "Wf": to32(Wf), "bf": _bias_cols(bf),
         "Wl1": to32(Wl1), "bl1": _bias_cols(bl1),
         "Wl2": to32(Wl2), "bl2": np.asarray(bl2, np.float32).reshape(1, 1),
         }
        for c in range(N_CORES)
    ]
    r3 = _run(nc3, in3, "l3")

    h_ci = np.zeros((N_NODES, HID), np.float32)
    s_ci = np.zeros((N_NODES,), np.float32)
    for c in range(N_CORES):
        nos = plan["cores"][c]["node_of_slot"]
        v = valid[c]
        h_ci[nos[v]] = r3[c]["hT"].T[v]
        s_ci[nos[v]] = r3[c]["yT"][0][v]

    return (s_ci, s_ci, s_ci, h_ci, h_ci)
